# revision 57
# baseline (speedup 1.0000x reference)
"""Self-contained Trainium2 kernel for nn_B3SplineUWT (3-level B3-spline
undecimated wavelet transform), data-parallel over 8 NeuronCores.

kernel(x: [8,1024,1024] f32) -> [8,4,1024,1024] f32  (w1,w2,w3,c3)

Per core: one image, bf16 internal data path (~5e-3 rel, tol 2e-2).
  - H-conv (partition dim): PE banded matmuls with raw integer taps
    (1,4,6), 3-block banded form; the 1/256 normalization (both
    separable passes) rides the ACT PSUM-evacuation scale for free.
  - W-conv: per-level split between PE (5 shifted accumulating matmuls,
    stationary w*I, the shift in the rhs AP offset) and DVE (4 chained
    (1 + z^d) bf16 tensor_adds -- the binomial factorization of the
    5-tap (1,4,6,4,1); even shifts for d=2,4 keep the DVE 2x mode).
  - w_j = c_{j-1} - c_j on DVE in bf16 into bf16 staging.
  - I/O via SWDGE cast-DMAs (f32->bf16 load, bf16->f32 store), streamed
    per 2 chunks (per chunk for c3) so the DMA drains during compute.
  - Per-chunk tiles, one buffer generation per level, 1-bank PSUM
    tiles, wavefront (level,chunk) emission order for cross-level
    pipelining.

Engine budget (cost model, per core): DVE ~52us (critical chain),
DMA ~55us, PE ~47us, ACT ~40us, Pool ~33us; end-to-end ~80us.
"""
import numpy as np

import concourse.bacc as bacc
import concourse.bass as bass
import concourse.mybir as mybir
import concourse.tile as tile
from concourse.bass_utils import run_bass_kernel_spmd

F32 = mybir.dt.float32
BF16 = mybir.dt.bfloat16
COPY = mybir.ActivationFunctionType.Copy

B = 8
H = 1024
W = 1024
P = 128
NCH = H // P
LEVELS = 3
DILS = (1, 2, 4)
MARG = 16           # left/right margin in yx (>= 2*max(d) = 8, 4B-aligned)
WE = W + 2 * MARG

# chunks whose W-conv runs on PE (rest on DVE), per level; tuned so both
# engines stay busy through each level's tail, with DVE getting early
# chunks (input arrives in chunk order)
PE_ROUTE = ({0, 4}, {2, 4, 6}, {2, 4, 6})

YX_BUFS = 6
CASC_BUFS = 3
WST_BUFS = 4
PSUM_BUFS = 8
WAVE_LAG = 3                  # chunk skew between consecutive levels
INPUT_GROUPS = (1, 1, 2, 2, 2)  # chunks per input cast-DMA
W_GROUP = (2, 2, 2)           # chunks per w_j output DMA, per level
W3_VIA_ACCUM = False          # w3 = c2 - c3 via CCE accum DMAs

TAPS = {0: 6.0, 1: 4.0, 2: 1.0}   # raw integer taps, exact in bf16
EVAC_SCALE = 1.0 / 256.0          # both 1/16 normalizations, on ACT evac


def _reflect(i, n):
    if i < 0:
        return -i
    if i >= n:
        return 2 * (n - 1) - i
    return i


def _build_blocks():
    """Per level: diagonal 128x128 blocks D[co] (reflect folded at the
    edges) and off-diagonal neighbor blocks for the banded H-conv."""
    per_level = []
    for d in DILS:
        full = np.zeros((H, H), np.float32)
        for r in range(H):
            for o in (-2 * d, -d, 0, d, 2 * d):
                full[_reflect(r + o, H), r] += TAPS[abs(o) // d]
        dblk, offdiag = [], []
        for co in range(NCH):
            r0 = co * P
            dblk.append(np.ascontiguousarray(full[r0:r0 + P, r0:r0 + P]))
            od = []
            for ci in (co - 1, co + 1):
                if 0 <= ci < NCH:
                    blk = full[ci * P:(ci + 1) * P, r0:r0 + P]
                    if np.any(blk != 0):
                        od.append((ci, np.ascontiguousarray(blk)))
            offdiag.append(od)
        per_level.append((dblk, offdiag))
    return per_level


def _pack_consts(per_level):
    mats, seen = [], {}

    def intern(m):
        h = m.tobytes()
        if h not in seen:
            seen[h] = len(mats) * P
            mats.append(m)
        return seen[h]

    index = []
    for dblk, offdiag in per_level:
        doffs = [intern(m) for m in dblk]
        ooffs = [[(ci, intern(m)) for ci, m in od] for od in offdiag]
        index.append((doffs, ooffs))
    ident_offs = {}
    for w in (1.0, 4.0, 6.0):
        ident_offs[w] = len(mats) * P
        mats.append(np.eye(P, dtype=np.float32) * w)
    packed = np.ascontiguousarray(
        np.concatenate(mats, axis=1).astype(np.float32))
    return packed, index, ident_offs


def _build_program():
    per_level = _build_blocks()
    consts_np, cindex, ident_offs = _pack_consts(per_level)
    ncols_const = consts_np.shape[1]

    nc = bacc.Bacc("TRN2", target_bir_lowering=False, debug=False)
    x_d = nc.dram_tensor("x", [H, W], F32, kind="ExternalInput")
    c_d = nc.dram_tensor("consts", [P, ncols_const], F32,
                         kind="ExternalInput")
    out_d = nc.dram_tensor("out", [LEVELS + 1, H, W], F32,
                           kind="ExternalOutput")

    with tile.TileContext(nc) as tc:
        with tc.tile_pool(name="sb", bufs=1) as sb, \
             tc.tile_pool(name="yxp", bufs=YX_BUFS) as yxp, \
             tc.tile_pool(name="casc", bufs=CASC_BUFS) as casc, \
             tc.tile_pool(name="wst", bufs=WST_BUFS) as wstp, \
             tc.tile_pool(name="ps", bufs=PSUM_BUFS, space="PSUM") as ps:

            # constants: f32 DRAM -> bf16 SBUF via SWDGE cast DMA
            cr = sb.tile([P, ncols_const], BF16, tag="cr", name="cr")
            nc.gpsimd.dma_start(cr[:], c_d[:])

            def wident(w):
                off = ident_offs[w]
                return cr[:, off:off + P]

            # input: f32 DRAM -> bf16 SBUF cast DMAs; single-chunk loads
            # up front so the first H-convs (and the DVE cascade chain
            # behind them) start as early as possible
            xq = []          # per-chunk accessor: (tile, idx_in_tile)
            for g, n in enumerate(INPUT_GROUPS):
                t = sb.tile([P, n, W], BF16, tag=f"xq{g}", name=f"xq{g}")
                base = sum(INPUT_GROUPS[:g])
                nc.gpsimd.dma_start(
                    t[:],
                    bass.AP(x_d, base * P * W,
                            [[W, P], [P * W, n], [1, W]]))
                for k in range(n):
                    xq.append((t, k))

            # per-chunk level buffers, one generation per level (no WARs)
            cbuf = [[sb.tile([P, W], BF16, tag=f"c{g}_{co}",
                             name=f"c{g}_{co}") for co in range(NCH)]
                    for g in range(LEVELS)]

            def chunk_in(j, co, lo=0, hi=W):
                if j == 0:
                    t, k = xq[co]
                    return t[:, k, lo:hi]
                return cbuf[j - 1][co][:, lo:hi]

            def cur_ap(j, co, lo=0, hi=W):
                return cbuf[j][co][:, lo:hi]

            # wavefront emission order: level j trails level j-1 by
            # WAVE_LAG chunks, so late-level outputs stream from
            # mid-kernel instead of piling into a DMA-only tail
            order = []
            for wave in range(NCH + WAVE_LAG * (LEVELS - 1)):
                for j in range(LEVELS):
                    co = wave - WAVE_LAG * j
                    if 0 <= co < NCH:
                        order.append((j, co))

            wsts = {}
            for j, co in order:
                d = DILS[j]
                doffs, ooffs = cindex[j]
                shifts = [(0, 6.0), (-d, 4.0), (d, 4.0),
                          (-2 * d, 1.0), (2 * d, 1.0)]

                # ---- H-conv into PSUM (banded matmuls, raw taps) ----
                yx = yxp.tile([P, WE], BF16, tag="yx", name="yx")
                for half in range(2):
                    lo, hi = half * 512, (half + 1) * 512
                    pt = ps.tile([P, 512], F32, tag="psum", name="pt",
                                 bufs=PSUM_BUFS)
                    mms = ([(doffs[co], None)] +
                           [(off, ci) for ci, off in ooffs[co]])
                    for i, (off, ci) in enumerate(mms):
                        nc.tensor.matmul(
                            pt[:], cr[:, off:off + P],
                            chunk_in(j, co if ci is None else ci, lo, hi),
                            start=(i == 0),
                            stop=(i == len(mms) - 1))
                    # evacuate with the 1/256 scale into the margin tile
                    nc.scalar.activation(
                        yx[:, MARG + lo:MARG + hi],
                        pt[:], COPY, scale=EVAC_SCALE)

                # reflect margins (Pool): yx[M-k] = yx[M+k]
                nc.gpsimd.tensor_copy(
                    bass.AP(yx.tensor, MARG - 2 * d, [[WE, P], [1, 2 * d]]),
                    bass.AP(yx.tensor, MARG + 2 * d, [[WE, P], [-1, 2 * d]]))
                nc.gpsimd.tensor_copy(
                    bass.AP(yx.tensor, MARG + W, [[WE, P], [1, 2 * d]]),
                    bass.AP(yx.tensor, MARG + W - 2, [[WE, P], [-1, 2 * d]]))

                # ---- W-conv ----
                if co in PE_ROUTE[j]:
                    # PE route: 5 shifted accumulating matmuls per half
                    for half in range(2):
                        pc = ps.tile([P, 512], F32, tag="psum",
                                     name="pc", bufs=PSUM_BUFS)
                        base = MARG + half * 512
                        for i, (off, wgt) in enumerate(shifts):
                            nc.tensor.matmul(
                                pc[:],
                                wident(wgt),
                                bass.AP(yx.tensor, base + off,
                                        [[WE, P], [1, 512]]),
                                start=(i == 0),
                                stop=(i == len(shifts) - 1))
                        nc.scalar.copy(
                            cur_ap(j, co, half * 512, (half + 1) * 512),
                            pc[:])
                else:
                    # DVE route: 4 chained (1 + z^d) adds, bf16 2x mode
                    def yxs(o, width):
                        return bass.AP(yx.tensor, MARG + o,
                                       [[WE, P], [1, width]])
                    t1 = casc.tile([P, WE], BF16, tag="t1", name="t1")
                    t2 = casc.tile([P, WE], BF16, tag="t2", name="t2")
                    w1 = W + 3 * d
                    nc.vector.tensor_add(
                        t1[:, :w1], yxs(-2 * d, w1), yxs(-d, w1))
                    w2 = W + 2 * d
                    nc.vector.tensor_add(
                        t2[:, :w2], t1[:, :w2], t1[:, d:d + w2])
                    w3 = W + d
                    nc.vector.tensor_add(
                        t1[:, :w3], t2[:, :w3], t2[:, d:d + w3])
                    nc.vector.tensor_add(
                        cur_ap(j, co), t1[:, :W], t1[:, d:d + W])

                # c3: stream each chunk as soon as its W-conv is done
                if j == LEVELS - 1:
                    nc.gpsimd.dma_start(
                        bass.AP(out_d, 3 * H * W + co * P * W,
                                [[W, P], [1, W]]),
                        cur_ap(j, co))

                if j == LEVELS - 1 and W3_VIA_ACCUM:
                    # w3 = c2 - c3 entirely in the DMA engines: write the
                    # c2 chunk into the w3 region, then accum-subtract c3
                    w3ap = bass.AP(out_d, j * H * W + co * P * W,
                                   [[W, P], [1, W]])
                    nc.gpsimd.dma_start(w3ap, chunk_in(j, co))
                    nc.gpsimd.dma_start(
                        w3ap, cur_ap(j, co),
                        accum_op=mybir.AluOpType.subtract)
                    continue

                # ---- w_j = prev - cur (bf16) into staging ----
                wg = W_GROUP[j]
                hv, ci_ = divmod(co, wg)
                if ci_ == 0:
                    wsts[(j, hv)] = wstp.tile([P, wg, W], BF16,
                                              tag="wst", name="wst")
                nc.vector.tensor_sub(
                    wsts[(j, hv)][:, ci_, :], chunk_in(j, co),
                    cur_ap(j, co))

                # ---- stream out per group (cast bf16->f32) ----
                if ci_ == wg - 1:
                    nc.gpsimd.dma_start(
                        bass.AP(out_d, j * H * W + hv * wg * P * W,
                                [[W, P], [P * W, wg], [1, W]]),
                        wsts[(j, hv)][:])

    nc.compile()
    return nc, consts_np


_CACHE = {}


def _get_program():
    if "prog" not in _CACHE:
        _CACHE["prog"] = _build_program()
    return _CACHE["prog"]


def kernel(x, _trace=False, _trace_kwargs=None):
    """x: [8, 1024, 1024] float32 -> [8, 4, 1024, 1024] float32."""
    x = np.asarray(x)
    assert x.shape == (B, H, W) and x.dtype == np.float32
    nc, consts_np = _get_program()
    in_maps = [{"x": np.ascontiguousarray(x[b]), "consts": consts_np}
               for b in range(B)]
    kw = {}
    if _trace:
        kw = dict(trace=True, **(_trace_kwargs or {}))
    res = run_bass_kernel_spmd(nc, in_maps, core_ids=list(range(B)), **kw)
    out = np.stack([r["out"] for r in res.results], axis=0)
    if _trace:
        return out, res
    return out


# revision 58
# speedup vs baseline: 1.0242x; 1.0242x over previous
"""Self-contained Trainium2 kernel for nn_B3SplineUWT (3-level B3-spline
undecimated wavelet transform), data-parallel over 8 NeuronCores.

kernel(x: [8,1024,1024] f32) -> [8,4,1024,1024] f32  (w1,w2,w3,c3)

Per core: one image, bf16 internal data path (~5e-3 rel, tol 2e-2).
  - H-conv (partition dim): PE banded matmuls with raw integer taps
    (1,4,6), 3-block banded form; the 1/256 normalization (both
    separable passes) rides the ACT PSUM-evacuation scale for free.
  - W-conv: per-level split between PE (5 shifted accumulating matmuls,
    stationary w*I, the shift in the rhs AP offset) and DVE (4 chained
    (1 + z^d) bf16 tensor_adds -- the binomial factorization of the
    5-tap (1,4,6,4,1); even shifts for d=2,4 keep the DVE 2x mode).
  - w_j = c_{j-1} - c_j on DVE in bf16 into bf16 staging.
  - I/O via SWDGE cast-DMAs (f32->bf16 load, bf16->f32 store), streamed
    per 2 chunks (per chunk for c3) so the DMA drains during compute.
  - Per-chunk tiles, one buffer generation per level, 1-bank PSUM
    tiles, wavefront (level,chunk) emission order for cross-level
    pipelining.

Engine budget (cost model, per core): DVE ~52us (critical chain),
DMA ~55us, PE ~47us, ACT ~40us, Pool ~33us; end-to-end ~80us.
"""
import numpy as np

import concourse.bacc as bacc
import concourse.bass as bass
import concourse.mybir as mybir
import concourse.tile as tile
from concourse.bass_utils import run_bass_kernel_spmd

F32 = mybir.dt.float32
BF16 = mybir.dt.bfloat16
COPY = mybir.ActivationFunctionType.Copy

B = 8
H = 1024
W = 1024
P = 128
NCH = H // P
LEVELS = 3
DILS = (1, 2, 4)
MARG = 16           # left/right margin in yx (>= 2*max(d) = 8, 4B-aligned)
WE = W + 2 * MARG

# chunks whose W-conv runs on PE (rest on DVE), per level; tuned by
# cost-model search so both engines stay busy through each level's tail
PE_ROUTE = ({7}, {0, 1, 6}, {1, 3, 6})

YX_BUFS = 6
CASC_BUFS = 3
WST_BUFS = 4
PSUM_BUFS = 8
WAVE_LAG = 4                  # chunk skew between consecutive levels
INPUT_GROUPS = (1, 1, 3, 3)   # chunks per input cast-DMA
W_GROUP = (2, 2, 2)           # chunks per w_j output DMA, per level
W3_VIA_ACCUM = False          # w3 = c2 - c3 via CCE accum DMAs

TAPS = {0: 6.0, 1: 4.0, 2: 1.0}   # raw integer taps, exact in bf16
EVAC_SCALE = 1.0 / 256.0          # both 1/16 normalizations, on ACT evac


def _reflect(i, n):
    if i < 0:
        return -i
    if i >= n:
        return 2 * (n - 1) - i
    return i


def _build_blocks():
    """Per level: diagonal 128x128 blocks D[co] (reflect folded at the
    edges) and off-diagonal neighbor blocks for the banded H-conv."""
    per_level = []
    for d in DILS:
        full = np.zeros((H, H), np.float32)
        for r in range(H):
            for o in (-2 * d, -d, 0, d, 2 * d):
                full[_reflect(r + o, H), r] += TAPS[abs(o) // d]
        dblk, offdiag = [], []
        for co in range(NCH):
            r0 = co * P
            dblk.append(np.ascontiguousarray(full[r0:r0 + P, r0:r0 + P]))
            od = []
            for ci in (co - 1, co + 1):
                if 0 <= ci < NCH:
                    blk = full[ci * P:(ci + 1) * P, r0:r0 + P]
                    if np.any(blk != 0):
                        od.append((ci, np.ascontiguousarray(blk)))
            offdiag.append(od)
        per_level.append((dblk, offdiag))
    return per_level


def _pack_consts(per_level):
    mats, seen = [], {}

    def intern(m):
        h = m.tobytes()
        if h not in seen:
            seen[h] = len(mats) * P
            mats.append(m)
        return seen[h]

    index = []
    for dblk, offdiag in per_level:
        doffs = [intern(m) for m in dblk]
        ooffs = [[(ci, intern(m)) for ci, m in od] for od in offdiag]
        index.append((doffs, ooffs))
    ident_offs = {}
    for w in (1.0, 4.0, 6.0):
        ident_offs[w] = len(mats) * P
        mats.append(np.eye(P, dtype=np.float32) * w)
    packed = np.ascontiguousarray(
        np.concatenate(mats, axis=1).astype(np.float32))
    return packed, index, ident_offs


def _build_program():
    per_level = _build_blocks()
    consts_np, cindex, ident_offs = _pack_consts(per_level)
    ncols_const = consts_np.shape[1]

    nc = bacc.Bacc("TRN2", target_bir_lowering=False, debug=False)
    x_d = nc.dram_tensor("x", [H, W], F32, kind="ExternalInput")
    c_d = nc.dram_tensor("consts", [P, ncols_const], F32,
                         kind="ExternalInput")
    out_d = nc.dram_tensor("out", [LEVELS + 1, H, W], F32,
                           kind="ExternalOutput")

    with tile.TileContext(nc) as tc:
        with tc.tile_pool(name="sb", bufs=1) as sb, \
             tc.tile_pool(name="yxp", bufs=YX_BUFS) as yxp, \
             tc.tile_pool(name="casc", bufs=CASC_BUFS) as casc, \
             tc.tile_pool(name="wst", bufs=WST_BUFS) as wstp, \
             tc.tile_pool(name="ps", bufs=PSUM_BUFS, space="PSUM") as ps:

            # constants: f32 DRAM -> bf16 SBUF via SWDGE cast DMA
            cr = sb.tile([P, ncols_const], BF16, tag="cr", name="cr")
            nc.gpsimd.dma_start(cr[:], c_d[:])

            def wident(w):
                off = ident_offs[w]
                return cr[:, off:off + P]

            # input: f32 DRAM -> bf16 SBUF cast DMAs; single-chunk loads
            # up front so the first H-convs (and the DVE cascade chain
            # behind them) start as early as possible
            xq = []          # per-chunk accessor: (tile, idx_in_tile)
            for g, n in enumerate(INPUT_GROUPS):
                t = sb.tile([P, n, W], BF16, tag=f"xq{g}", name=f"xq{g}")
                base = sum(INPUT_GROUPS[:g])
                nc.gpsimd.dma_start(
                    t[:],
                    bass.AP(x_d, base * P * W,
                            [[W, P], [P * W, n], [1, W]]))
                for k in range(n):
                    xq.append((t, k))

            # per-chunk level buffers, one generation per level (no WARs)
            cbuf = [[sb.tile([P, W], BF16, tag=f"c{g}_{co}",
                             name=f"c{g}_{co}") for co in range(NCH)]
                    for g in range(LEVELS)]

            def chunk_in(j, co, lo=0, hi=W):
                if j == 0:
                    t, k = xq[co]
                    return t[:, k, lo:hi]
                return cbuf[j - 1][co][:, lo:hi]

            def cur_ap(j, co, lo=0, hi=W):
                return cbuf[j][co][:, lo:hi]

            # wavefront emission order: level j trails level j-1 by
            # WAVE_LAG chunks, so late-level outputs stream from
            # mid-kernel instead of piling into a DMA-only tail
            order = []
            for wave in range(NCH + WAVE_LAG * (LEVELS - 1)):
                for j in range(LEVELS):
                    co = wave - WAVE_LAG * j
                    if 0 <= co < NCH:
                        order.append((j, co))

            wsts = {}
            for j, co in order:
                d = DILS[j]
                doffs, ooffs = cindex[j]
                shifts = [(0, 6.0), (-d, 4.0), (d, 4.0),
                          (-2 * d, 1.0), (2 * d, 1.0)]

                # ---- H-conv into PSUM (banded matmuls, raw taps) ----
                yx = yxp.tile([P, WE], BF16, tag="yx", name="yx")
                for half in range(2):
                    lo, hi = half * 512, (half + 1) * 512
                    pt = ps.tile([P, 512], F32, tag="psum", name="pt",
                                 bufs=PSUM_BUFS)
                    mms = ([(doffs[co], None)] +
                           [(off, ci) for ci, off in ooffs[co]])
                    for i, (off, ci) in enumerate(mms):
                        nc.tensor.matmul(
                            pt[:], cr[:, off:off + P],
                            chunk_in(j, co if ci is None else ci, lo, hi),
                            start=(i == 0),
                            stop=(i == len(mms) - 1))
                    # evacuate with the 1/256 scale into the margin tile
                    nc.scalar.activation(
                        yx[:, MARG + lo:MARG + hi],
                        pt[:], COPY, scale=EVAC_SCALE)

                # reflect margins (Pool): yx[M-k] = yx[M+k]
                nc.gpsimd.tensor_copy(
                    bass.AP(yx.tensor, MARG - 2 * d, [[WE, P], [1, 2 * d]]),
                    bass.AP(yx.tensor, MARG + 2 * d, [[WE, P], [-1, 2 * d]]))
                nc.gpsimd.tensor_copy(
                    bass.AP(yx.tensor, MARG + W, [[WE, P], [1, 2 * d]]),
                    bass.AP(yx.tensor, MARG + W - 2, [[WE, P], [-1, 2 * d]]))

                # ---- W-conv ----
                if co in PE_ROUTE[j]:
                    # PE route: 5 shifted accumulating matmuls per half
                    for half in range(2):
                        pc = ps.tile([P, 512], F32, tag="psum",
                                     name="pc", bufs=PSUM_BUFS)
                        base = MARG + half * 512
                        for i, (off, wgt) in enumerate(shifts):
                            nc.tensor.matmul(
                                pc[:],
                                wident(wgt),
                                bass.AP(yx.tensor, base + off,
                                        [[WE, P], [1, 512]]),
                                start=(i == 0),
                                stop=(i == len(shifts) - 1))
                        nc.scalar.copy(
                            cur_ap(j, co, half * 512, (half + 1) * 512),
                            pc[:])
                else:
                    # DVE route: 4 chained (1 + z^d) adds, bf16 2x mode
                    def yxs(o, width):
                        return bass.AP(yx.tensor, MARG + o,
                                       [[WE, P], [1, width]])
                    t1 = casc.tile([P, WE], BF16, tag="t1", name="t1")
                    t2 = casc.tile([P, WE], BF16, tag="t2", name="t2")
                    w1 = W + 3 * d
                    nc.vector.tensor_add(
                        t1[:, :w1], yxs(-2 * d, w1), yxs(-d, w1))
                    w2 = W + 2 * d
                    nc.vector.tensor_add(
                        t2[:, :w2], t1[:, :w2], t1[:, d:d + w2])
                    w3 = W + d
                    nc.vector.tensor_add(
                        t1[:, :w3], t2[:, :w3], t2[:, d:d + w3])
                    nc.vector.tensor_add(
                        cur_ap(j, co), t1[:, :W], t1[:, d:d + W])

                # c3: stream each chunk as soon as its W-conv is done
                if j == LEVELS - 1:
                    nc.gpsimd.dma_start(
                        bass.AP(out_d, 3 * H * W + co * P * W,
                                [[W, P], [1, W]]),
                        cur_ap(j, co))

                if j == LEVELS - 1 and W3_VIA_ACCUM:
                    # w3 = c2 - c3 entirely in the DMA engines: write the
                    # c2 chunk into the w3 region, then accum-subtract c3
                    w3ap = bass.AP(out_d, j * H * W + co * P * W,
                                   [[W, P], [1, W]])
                    nc.gpsimd.dma_start(w3ap, chunk_in(j, co))
                    nc.gpsimd.dma_start(
                        w3ap, cur_ap(j, co),
                        accum_op=mybir.AluOpType.subtract)
                    continue

                # ---- w_j = prev - cur (bf16) into staging ----
                wg = W_GROUP[j]
                hv, ci_ = divmod(co, wg)
                if ci_ == 0:
                    wsts[(j, hv)] = wstp.tile([P, wg, W], BF16,
                                              tag="wst", name="wst")
                nc.vector.tensor_sub(
                    wsts[(j, hv)][:, ci_, :], chunk_in(j, co),
                    cur_ap(j, co))

                # ---- stream out per group (cast bf16->f32) ----
                if ci_ == wg - 1:
                    nc.gpsimd.dma_start(
                        bass.AP(out_d, j * H * W + hv * wg * P * W,
                                [[W, P], [P * W, wg], [1, W]]),
                        wsts[(j, hv)][:])

    nc.compile()
    return nc, consts_np


_CACHE = {}


def _get_program():
    if "prog" not in _CACHE:
        _CACHE["prog"] = _build_program()
    return _CACHE["prog"]


def kernel(x, _trace=False, _trace_kwargs=None):
    """x: [8, 1024, 1024] float32 -> [8, 4, 1024, 1024] float32."""
    x = np.asarray(x)
    assert x.shape == (B, H, W) and x.dtype == np.float32
    nc, consts_np = _get_program()
    in_maps = [{"x": np.ascontiguousarray(x[b]), "consts": consts_np}
               for b in range(B)]
    kw = {}
    if _trace:
        kw = dict(trace=True, **(_trace_kwargs or {}))
    res = run_bass_kernel_spmd(nc, in_maps, core_ids=list(range(B)), **kw)
    out = np.stack([r["out"] for r in res.results], axis=0)
    if _trace:
        return out, res
    return out
